# revision 1
# baseline (speedup 1.0000x reference)
"""Trainium2 Bass kernel for nn_MultiHeadCoAttention.

B=32, LT=512, LI=576, D=768, H=8, K=512. Batch-parallel over 8 cores
(4 batches per core, no collectives). All matmuls in float32r (rne-11
rounding, full PE rate at N>=256).
"""
import sys
sys.path.insert(0, '/opt/trn_rl_repo')
import numpy as np
import concourse.bacc as bacc
import concourse.tile as tile
from concourse import bass, mybir
from concourse.bass_utils import run_bass_kernel_spmd

F32 = mybir.dt.float32
F32R = mybir.dt.float32r
AF = mybir.ActivationFunctionType
OP = mybir.AluOpType

B, LT, LI, D, H, K = 32, 512, 576, 768, 8, 512
NB = 4           # batches per core
N_CORES = 8
ET = D // 128    # 6 e-tiles
XT = LT // 128   # 4 x-tiles
YT = 5           # y-tiles (4 full + 1 of 64)
YP = [128, 128, 128, 128, 64]


def ycols(j):
    return 128 if j < 4 else 64


def build_nc(repeat=1, skip=(), bufs_overrides=None, preadd=False, gvlate=False):
    bo_ = bufs_overrides or {}
    nc = bacc.Bacc(None, target_bir_lowering=False)

    # ---- DRAM I/O (per core) ----
    textT = nc.dram_tensor("textT", [NB, D, LT], F32, kind="ExternalInput")
    text_aug = nc.dram_tensor("text_aug", [NB, LT, D + 4], F32, kind="ExternalInput")
    imageT = nc.dram_tensor("imageT", [NB, D, LI], F32, kind="ExternalInput")
    image_aug = nc.dram_tensor("image_aug", [NB, LI, D + 4], F32, kind="ExternalInput")
    WqT_d = nc.dram_tensor("WqT", [D, K], F32, kind="ExternalInput")
    WvT_d = nc.dram_tensor("WvT", [D, K], F32, kind="ExternalInput")
    WbT_d = nc.dram_tensor("WbT", [H, D, D], F32, kind="ExternalInput")
    WhvB_d = nc.dram_tensor("WhvB", [128, K], F32, kind="ExternalInput")
    WhqB_d = nc.dram_tensor("WhqB", [128, K], F32, kind="ExternalInput")
    WoT_d = nc.dram_tensor("WoT", [H * D, D], F32, kind="ExternalInput")
    ident_d = nc.dram_tensor("ident", [128, 128], F32, kind="ExternalInput")
    bo_d = nc.dram_tensor("bo_rep", [NB, D], F32, kind="ExternalInput")
    out_d = nc.dram_tensor("out", [NB, D], F32, kind="ExternalOutput")

    with tile.TileContext(nc) as tc:
        with (
            tc.tile_pool(name="const", bufs=1) as const,
            tc.tile_pool(name="perb", bufs=1) as perb,
            tc.tile_pool(name="wbt", bufs=bo_.get("wbt") or 2) as wbtp,
            tc.tile_pool(name="stage", bufs=bo_.get("stage") or 4) as stagep,
            tc.tile_pool(name="ctxrhs", bufs=bo_.get("ctxrhs") or 2) as ctxp,
            tc.tile_pool(name="ptp", bufs=bo_.get("ptp") or 1) as ptp,
            tc.tile_pool(name="affp", bufs=bo_.get("affp") or 1) as affp,
            tc.tile_pool(name="hch", bufs=bo_.get("hch") or 2) as hchp,
            tc.tile_pool(name="prod", bufs=bo_.get("prod") or 2) as prodp,
            tc.tile_pool(name="wot", bufs=bo_.get("wot") or 3) as wotp,
            tc.tile_pool(name="psA", bufs=2, space="PSUM") as psA,
            tc.tile_pool(name="psB", bufs=2, space="PSUM") as psB,
            tc.tile_pool(name="psC", bufs=2, space="PSUM") as psC,
        ):
            def stage_tile():
                return stagep.tile([128, 772], F32, tag="stage", name="stage")

            def round_to(dst_ap, dram_ap, p=128):
                """DMA f32 dram -> stage, gpsimd-round -> f32r dest ap."""
                st = stage_tile()
                w = dram_ap.shape[-1]
                nc.sync.dma_start(st[0:p, 0:w], dram_ap)
                nc.gpsimd.tensor_copy(dst_ap, st[0:p, 0:w])

            # ---- constants ----
            wqt = const.tile([128, ET * K], F32R, tag="wqt")
            wvt = const.tile([128, ET * K], F32R, tag="wvt")
            for j in range(ET):
                round_to(wqt[:, j * K:(j + 1) * K], WqT_d[j * 128:(j + 1) * 128, :])
                round_to(wvt[:, j * K:(j + 1) * K], WvT_d[j * 128:(j + 1) * 128, :])
            whvb = const.tile([128, K], F32, tag="whvb")
            whqb = const.tile([128, K], F32, tag="whqb")
            nc.sync.dma_start(whvb[:], WhvB_d[:])
            nc.sync.dma_start(whqb[:], WhqB_d[:])
            ident = const.tile([128, 128], F32, tag="ident")
            nc.sync.dma_start(ident[:], ident_d[:])
            ident_r = const.tile([128, 128], F32R, tag="identr")
            nc.vector.tensor_copy(ident_r[:], ident[:])
            bo_t = const.tile([NB, D], F32, tag="bo")
            nc.sync.dma_start(bo_t[:], bo_d[:])
            # TComb col layout: c*32 + h*4 + b
            tcomb = const.tile([128, ET * H * NB], F32R, tag="tcomb")

            import contextlib
            loop_cm = tc.For_i(0, repeat, 1) if repeat > 1 else contextlib.nullcontext()
            with loop_cm:
              for b in range(NB):
                  # ---- stage + round per-batch tensors ----
                  tet = perb.tile([128, ET * LT], F32R, tag="tet")
                  iet = perb.tile([128, ET * LI], F32R, tag="iet")
                  for j in range(ET):
                      round_to(tet[:, j * LT:(j + 1) * LT], textT[b, j * 128:(j + 1) * 128, :])
                      round_to(iet[:, j * LI:(j + 1) * LI], imageT[b, j * 128:(j + 1) * 128, :])

                  # ---- wq_q [x,k] ----
                  wqq = perb.tile([128, XT * K], F32R, tag="wqq")
                  for i in range(XT):
                      ps = psA.tile([128, K], F32, tag="mm1")
                      for j in range(ET):
                          nc.tensor.matmul(
                              ps[:], tet[:, j * LT + i * 128: j * LT + (i + 1) * 128],
                              wqt[:, j * K:(j + 1) * K],
                              start=(j == 0), stop=(j == ET - 1))
                      nc.vector.tensor_copy(wqq[:, i * K:(i + 1) * K], ps[:])

                  # ---- wv_v [y,k] ----
                  wvv = perb.tile([128, YT * K], F32R, tag="wvv")
                  for j_y in range(YT):
                      p = ycols(j_y)
                      ps = psA.tile([128, K], F32, tag="mm1")
                      for j in range(ET):
                          nc.tensor.matmul(
                              ps[0:p, :],
                              iet[:, j * LI + j_y * 128: j * LI + j_y * 128 + p],
                              wvt[:, j * K:(j + 1) * K],
                              start=(j == 0), stop=(j == ET - 1))
                      nc.vector.tensor_copy(wvv[0:p, j_y * K:(j_y + 1) * K], ps[0:p, :])

                  # ---- G_v [e,k] = image.T @ wv_v  (3 passes of 2 d-chunks) ----
                  gv = perb.tile([128, ET * K], F32R, tag="gv")

                  def emit_gv(b=b, gv=gv, wvv=wvv):
                      if "gv" in skip:
                          for zi in range(4):
                              zz = stage_tile()
                              nc.vector.memset(zz[:, 0:768], 0.0)
                              nc.vector.tensor_copy(gv[:, zi * 768:(zi + 1) * 768], zz[:, 0:768])
                          return
                      for pr in range(3):
                          pss = [psA.tile([128, K], F32, tag="mm1", name="psgv") for _ in range(2)]
                          for j_y in range(YT):
                              p = ycols(j_y)
                              ch = ctxp.tile([128, D + 4], F32R, tag="ctxrhs")
                              round_to(ch[0:p, :], image_aug[b, j_y * 128:j_y * 128 + p, :], p=p)
                              for ci in range(2):
                                  c = 2 * pr + ci
                                  nc.tensor.matmul(
                                      pss[ci][0:128, :],
                                      ch[0:p, c * 128:(c + 1) * 128],
                                      wvv[0:p, j_y * K:(j_y + 1) * K],
                                      start=(j_y == 0), stop=(j_y == YT - 1))
                          for ci in range(2):
                              c = 2 * pr + ci
                              nc.vector.tensor_copy(gv[:, c * K:(c + 1) * K], pss[ci][:])

                  if not gvlate:
                      emit_gv()

                  sv = perb.tile([128, YT * H], F32, tag="sv")
                  sq = perb.tile([128, XT * H], F32, tag="sq")
                  nc.vector.memset(sv[:], 0.0)
                  nc.vector.memset(sq[:], 0.0)

                  # ---- heads ----
                  for h in range(H):
                      wbt = wbtp.tile([128, ET * D], F32R, tag="wbt")
                      for j in range(ET):
                          round_to(wbt[:, j * D:(j + 1) * D], WbT_d[h, j * 128:(j + 1) * 128, :])

                      # ptT [f,x] = Wb[h] @ text.T
                      ptt = ptp.tile([128, ET * LT], F32R, tag="ptt")
                      for c in range(ET):
                          ps = psA.tile([128, LT], F32, tag="mm1")
                          for j in range(ET):
                              nc.tensor.matmul(
                                  ps[:], wbt[:, j * D + c * 128: j * D + (c + 1) * 128],
                                  tet[:, j * LT:(j + 1) * LT],
                                  start=(j == 0), stop=(j == ET - 1))
                          nc.vector.tensor_copy(ptt[:, c * LT:(c + 1) * LT], ps[:])

                      # aff [x,y] = pt @ image.T   (y split 288+288)
                      aft = affp.tile([128, XT * LI], F32R, tag="aff")
                      for i in range(XT):
                          ps = psB.tile([128, 1024], F32, tag="aff")
                          for j in range(ET):
                              lhs = ptt[:, j * LT + i * 128: j * LT + (i + 1) * 128]
                              nc.tensor.matmul(
                                  ps[:, 0:288], lhs,
                                  iet[:, j * LI: j * LI + 288],
                                  start=(j == 0), stop=(j == ET - 1))
                              nc.tensor.matmul(
                                  ps[:, 512:800], lhs,
                                  iet[:, j * LI + 288: j * LI + 576],
                                  start=(j == 0), stop=(j == ET - 1))
                          ps3 = ps[:].rearrange("p (two x) -> p two x", two=2)[:, :, 0:288]
                          af3 = aft[:, i * LI:(i + 1) * LI].rearrange("p (two x) -> p two x", two=2)
                          nc.vector.tensor_copy(af3, ps3)

                      if gvlate and h == 0:
                          emit_gv()

                      # wqqc [y,k] + wv_v -> tanh -> *Whv -> reduce -> S_v
                      for j_y in range(YT):
                          p = ycols(j_y)
                          ps = psC.tile([128, K], F32, tag="pre")
                          if preadd:
                              nc.tensor.matmul(
                                  ps[0:p, :], ident_r[0:p, 0:p],
                                  wvv[0:p, j_y * K:(j_y + 1) * K],
                                  start=True, stop=False)
                          for i in range(XT):
                              nc.tensor.matmul(
                                  ps[0:p, :],
                                  aft[:, i * LI + j_y * 128: i * LI + j_y * 128 + p],
                                  wqq[:, i * K:(i + 1) * K],
                                  start=(i == 0) and not preadd, stop=(i == XT - 1))
                          if "post" in skip:
                              nc.vector.tensor_scalar_mul(sv[0:p, j_y * H + h: j_y * H + h + 1], ps[0:p, 0:1], 0.0)
                              continue
                          if not preadd:
                              nc.vector.tensor_tensor(
                                  out=ps[0:p, :], in0=ps[0:p, :],
                                  in1=wvv[0:p, j_y * K:(j_y + 1) * K].bitcast(F32), op=OP.add)
                          hc = hchp.tile([128, K], F32, tag="hch")
                          nc.scalar.activation(hc[0:p, :], ps[0:p, :], AF.Tanh)
                          pd = prodp.tile([128, K], F32, tag="prod")
                          nc.vector.tensor_tensor(
                              out=pd[0:p, :], in0=hc[0:p, :], in1=whvb[0:p, :], op=OP.mult)
                          nc.vector.tensor_reduce(
                              sv[0:p, j_y * H + h: j_y * H + h + 1], pd[0:p, :],
                              axis=mybir.AxisListType.X, op=OP.add)

                      # wvvc [x,k] + wq_q -> tanh -> *Whq -> reduce -> S_q
                      for i in range(XT):
                          ps = psC.tile([128, K], F32, tag="pre")
                          if preadd:
                              nc.tensor.matmul(
                                  ps[:], ident_r[:],
                                  wqq[:, i * K:(i + 1) * K],
                                  start=True, stop=False)
                          for c in range(ET):
                              nc.tensor.matmul(
                                  ps[:],
                                  ptt[:, c * LT + i * 128: c * LT + (i + 1) * 128],
                                  gv[:, c * K:(c + 1) * K],
                                  start=(c == 0) and not preadd, stop=(c == ET - 1))
                          if "post" in skip:
                              nc.vector.tensor_scalar_mul(sq[:, i * H + h: i * H + h + 1], ps[:, 0:1], 0.0)
                              continue
                          if not preadd:
                              nc.vector.tensor_tensor(
                                  out=ps[:], in0=ps[:],
                                  in1=wqq[:, i * K:(i + 1) * K].bitcast(F32), op=OP.add)
                          hc = hchp.tile([128, K], F32, tag="hch")
                          nc.scalar.activation(hc[:], ps[:], AF.Tanh)
                          pd = prodp.tile([128, K], F32, tag="prod")
                          nc.vector.tensor_tensor(out=pd[:], in0=hc[:], in1=whqb[:], op=OP.mult)
                          nc.vector.tensor_reduce(
                              sq[:, i * H + h: i * H + h + 1], pd[:],
                              axis=mybir.AxisListType.X, op=OP.add)

                  # ---- softmax numerators (no max-sub; logits bounded) ----
                  evf = perb.tile([128, YT * H], F32, tag="evf")
                  eqf = perb.tile([128, XT * H], F32, tag="eqf")
                  nc.scalar.activation(evf[:], sv[:], AF.Exp)
                  nc.scalar.activation(eqf[:], sq[:], AF.Exp)
                  evr = perb.tile([128, YT * H], F32R, tag="evr")
                  eqr = perb.tile([128, XT * H], F32R, tag="eqr")
                  nc.vector.tensor_copy(evr[:], evf[:])
                  nc.vector.tensor_copy(eqr[:], eqf[:])

                  # ---- ctx_v = E_v.T @ [image|1] ; ctx_q = E_q.T @ [text|1] ----
                  ho = perb.tile([H, D], F32, tag="ho")
                  tmph = perb.tile([H, D], F32, tag="tmph")
                  if "ctx" in skip:
                      nc.vector.memset(ho[:], 0.0)
                      nc.vector.memset(tmph[:], 0.0)
                  for (er, aug_d, nt, dest) in () if "ctx" in skip else (
                      (evr, image_aug, YT, ho),
                      (eqr, text_aug, XT, tmph),
                  ):
                      c512 = psA.tile([H, 512], F32, tag="mm1")
                      c257 = psA.tile([H, 260], F32, tag="mm1")
                      for j in range(nt):
                          p = ycols(j) if nt == YT else 128
                          ch = ctxp.tile([128, D + 4], F32R, tag="ctxrhs")
                          round_to(ch[0:p, :], aug_d[b, j * 128:j * 128 + p, :], p=p)
                          lhs = er[0:p, j * H:(j + 1) * H]
                          nc.tensor.matmul(c512[:], lhs, ch[0:p, 0:512],
                                           start=(j == 0), stop=(j == nt - 1))
                          nc.tensor.matmul(c257[:], lhs, ch[0:p, 512:772],
                                           start=(j == 0), stop=(j == nt - 1))
                      rcp = perb.tile([H, 1], F32, tag="rcp" + ("v" if dest is ho else "q"))
                      nc.vector.reciprocal(rcp[:], c257[:, 256:257])
                      nc.vector.tensor_scalar_mul(dest[:, 0:512], c512[:], rcp[:])
                      nc.vector.tensor_scalar_mul(dest[:, 512:768], c257[:, 0:256], rcp[:])
                  nc.vector.tensor_tensor(out=ho[:], in0=ho[:], in1=tmph[:], op=OP.add)

                  # ---- transpose head_out into TComb (col = c*32 + h*4 + b) ----
                  tc3 = tcomb[:].rearrange("p (c h b) -> p c h b", c=ET, h=H)
                  for c in range(ET):
                      pst = psA.tile([128, H], F32, tag="mm1")
                      nc.tensor.transpose(pst[:], ho[:, c * 128:(c + 1) * 128], ident[0:H, 0:H])
                      nc.vector.tensor_copy(tc3[:, c, :, b], pst[:])

              # ---- final: out = Wo @ combined + bo ----
              out_t = perb.tile([NB, D], F32, tag="outt")
              if "final" in skip:
                  nc.vector.memset(out_t[:], 0.0)
              else:
                  f512 = psA.tile([NB, 512], F32, tag="mm1")
                  f256 = psA.tile([NB, 256], F32, tag="mm1")
                  tc3 = tcomb[:].rearrange("p (c h b) -> p c h b", c=ET, h=H)
                  for t in range(H * ET):
                      h, c = t // ET, t % ET
                      wo = wotp.tile([128, D], F32R, tag="wot")
                      round_to(wo[:], WoT_d[t * 128:(t + 1) * 128, :])
                      lhs = tc3[:, c, h, :]
                      nc.tensor.matmul(f512[:], lhs, wo[:, 0:512],
                                       start=(t == 0), stop=(t == H * ET - 1))
                      nc.tensor.matmul(f256[:], lhs, wo[:, 512:768],
                                       start=(t == 0), stop=(t == H * ET - 1))
                  nc.vector.tensor_tensor(out=out_t[:, 0:512], in0=f512[:], in1=bo_t[:, 0:512], op=OP.add)
                  nc.vector.tensor_tensor(out=out_t[:, 512:768], in0=f256[:], in1=bo_t[:, 512:768], op=OP.add)
              nc.sync.dma_start(out_d[:], out_t[:])

    nc.compile()
    return nc


_nc_cache = None


def _get_nc():
    global _nc_cache
    if _nc_cache is None:
        _nc_cache = build_nc()
    return _nc_cache


def make_in_maps(inputs):
    return _make_in_maps(**inputs)


def _make_in_maps(text_hidden_states, image_hidden_states, text_mask, Wb, Wv,
                  Wq, Whv, Whq, Wo, bo, **_unused):
    text = np.ascontiguousarray(np.asarray(text_hidden_states, np.float32))
    image = np.ascontiguousarray(np.asarray(image_hidden_states, np.float32))
    Wb = np.asarray(Wb, np.float32)
    Wv = np.asarray(Wv, np.float32)
    Wq = np.asarray(Wq, np.float32)
    Whv = np.asarray(Whv, np.float32)
    Whq = np.asarray(Whq, np.float32)
    Wo = np.asarray(Wo, np.float32)
    bo = np.asarray(bo, np.float32)

    WqT = np.ascontiguousarray(Wq.T)
    WvT = np.ascontiguousarray(Wv.T)
    WbT = np.ascontiguousarray(np.transpose(Wb, (0, 2, 1)))
    WoT = np.ascontiguousarray(Wo.T)
    WhvB = np.ascontiguousarray(np.broadcast_to(Whv[None, :], (128, K)))
    WhqB = np.ascontiguousarray(np.broadcast_to(Whq[None, :], (128, K)))
    ident = np.eye(128, dtype=np.float32)

    textT = np.ascontiguousarray(np.transpose(text, (0, 2, 1)))
    imageT = np.ascontiguousarray(np.transpose(image, (0, 2, 1)))
    pad_t = np.zeros((B, LT, 4), np.float32); pad_t[:, :, 0] = 1.0
    pad_i = np.zeros((B, LI, 4), np.float32); pad_i[:, :, 0] = 1.0
    text_aug = np.concatenate([text, pad_t], axis=2)
    image_aug = np.concatenate([image, pad_i], axis=2)

    in_maps = []
    for c in range(N_CORES):
        sl = slice(c * NB, (c + 1) * NB)
        in_maps.append({
            "textT": textT[sl], "text_aug": text_aug[sl],
            "imageT": imageT[sl], "image_aug": image_aug[sl],
            "WqT": WqT, "WvT": WvT, "WbT": WbT,
            "WhvB": WhvB, "WhqB": WhqB, "WoT": WoT,
            "ident": ident,
            "bo_rep": np.ascontiguousarray(np.broadcast_to(bo[None, :], (NB, D))),
        })
    return in_maps


def kernel(**inputs):
    nc = _get_nc()
    in_maps = make_in_maps(inputs)
    r = run_bass_kernel_spmd(nc, in_maps, list(range(N_CORES)))
    return np.concatenate([r.results[c]["out"] for c in range(N_CORES)], axis=0)


if __name__ == "__main__":
    d = np.load("/root/problem/inputs_cache.npz")
    out = kernel(**{k: d[k] for k in d.files})
    exp = np.load("/root/problem/exp64.npy")
    err = np.abs(out.astype(np.float64) - exp)
    print("absmax err:", err.max(), "rel:", err.max() / np.abs(exp).max())



# revision 2
# speedup vs baseline: 1.0357x; 1.0357x over previous
"""Trainium2 Bass kernel for nn_MultiHeadCoAttention — v2 (fp16).

B=32, LT=512, LI=576, D=768, H=8, K=512. Batch-parallel over 8 cores
(4 batches per core, no collectives).

v2 vs v1 (f32r + gpsimd rounding everywhere):
- All matmul operands in fp16 (10-bit mantissa, full PE rate, DMA-native
  so no rounding passes; half the DMA traffic and SBUF of f32).
- tanh argument built in PSUM (matmul accumulation + DVE add), tanh on
  the scalar engine reading PSUM directly, writing fp16.
- (tanh * Whv) -> sum fused into one DVE tensor_tensor_reduce (fp16,
  2x DVE mode).
- PSUM->SBUF copies spread across DVE / ACT / Pool engines.
- Batched multi-tile DMAs; software-pipelined per-batch epilogue
  (softmax/ctx/transpose emitted under the next batch's matmuls);
  next-batch staging prefetched during the previous batch's heads.
"""
import sys
sys.path.insert(0, '/opt/trn_rl_repo')
import numpy as np
import concourse.bacc as bacc
import concourse.tile as tile
from concourse import bass, mybir
from concourse.bass_utils import run_bass_kernel_spmd

F32 = mybir.dt.float32
F16 = mybir.dt.float16
AF = mybir.ActivationFunctionType
OP = mybir.AluOpType

B, LT, LI, D, H, K = 32, 512, 576, 768, 8, 512
NB = 4           # batches per core
N_CORES = 8
ET = D // 128    # 6 e-tiles
XT = LT // 128   # 4 x-tiles
YT = 5           # y-tiles (4 full + 1 of 64)


def ycols(j):
    return 128 if j < 4 else 64


def build_nc(repeat=1, add_eng="dve", aff_eng="act", ptt_eng="act",
             psc_bufs=3, psb_bufs=1, wo_eng="sync", f16T=True, dma_multi=True,
             use_ttr=False):
    nc = bacc.Bacc(None, target_bir_lowering=False)

    # ---- DRAM I/O (per core), all fp16 except the f32 output ----
    textT = nc.dram_tensor("textT", [NB, D, LT], F16, kind="ExternalInput")
    text_aug = nc.dram_tensor("text_aug", [NB, LT, D + 4], F16, kind="ExternalInput")
    imageT = nc.dram_tensor("imageT", [NB, D, LI], F16, kind="ExternalInput")
    image_aug = nc.dram_tensor("image_aug", [NB, LI, D + 4], F16, kind="ExternalInput")
    WqT_d = nc.dram_tensor("WqT", [D, K], F16, kind="ExternalInput")
    WvT_d = nc.dram_tensor("WvT", [D, K], F16, kind="ExternalInput")
    WbT_d = nc.dram_tensor("WbT", [H, D, D], F16, kind="ExternalInput")
    WhvB_d = nc.dram_tensor("WhvB", [128, K], F16, kind="ExternalInput")
    WhqB_d = nc.dram_tensor("WhqB", [128, K], F16, kind="ExternalInput")
    WoT_d = nc.dram_tensor("WoT", [H * D, D], F16, kind="ExternalInput")
    ident_d = nc.dram_tensor("ident", [128, 128], F16, kind="ExternalInput")
    bo_d = nc.dram_tensor("bo_rep", [1, D], F16, kind="ExternalInput")
    ones_d = nc.dram_tensor("ones4", [1, NB], F16, kind="ExternalInput")
    out_d = nc.dram_tensor("out", [NB, D], F32, kind="ExternalOutput")

    with tile.TileContext(nc) as tc:
        with (
            tc.tile_pool(name="const", bufs=1) as const,
            tc.tile_pool(name="inp", bufs=2) as inp,
            tc.tile_pool(name="perb", bufs=1) as perb,
            tc.tile_pool(name="sxp", bufs=2) as sxp,        # sv/sq
            tc.tile_pool(name="taugp", bufs=2) as taugp,    # text_aug, resident
            tc.tile_pool(name="wbt", bufs=2) as wbtp,
            tc.tile_pool(name="ptp", bufs=2) as ptp,
            tc.tile_pool(name="affp", bufs=2) as affp,
            tc.tile_pool(name="hch", bufs=3) as hchp,
            tc.tile_pool(name="wot", bufs=4) as wotp,
            tc.tile_pool(name="psA", bufs=2, space="PSUM") as psA,
            tc.tile_pool(name="psB", bufs=psb_bufs, space="PSUM") as psB,
            tc.tile_pool(name="psC", bufs=psc_bufs, space="PSUM") as psC,
        ):
            def copy_eng(eng):
                if eng == "dve":
                    return nc.vector.tensor_copy
                if eng == "act":
                    return nc.scalar.copy
                return nc.gpsimd.tensor_copy

            # ---- constants (outside the timing loop) ----
            wqt = const.tile([128, ET * K], F16, tag="wqt")
            wvt = const.tile([128, ET * K], F16, tag="wvt")
            if dma_multi:
                nc.sync.dma_start(
                    wqt[:].rearrange("p (j k) -> p j k", j=ET),
                    WqT_d[:].rearrange("(j p) k -> p j k", p=128))
                nc.sync.dma_start(
                    wvt[:].rearrange("p (j k) -> p j k", j=ET),
                    WvT_d[:].rearrange("(j p) k -> p j k", p=128))
            else:
                for j in range(ET):
                    nc.sync.dma_start(wqt[:, j * K:(j + 1) * K],
                                      WqT_d[j * 128:(j + 1) * 128, :])
                    nc.sync.dma_start(wvt[:, j * K:(j + 1) * K],
                                      WvT_d[j * 128:(j + 1) * 128, :])
            whvb = const.tile([128, K], F16, tag="whvb")
            whqb = const.tile([128, K], F16, tag="whqb")
            nc.sync.dma_start(whvb[:], WhvB_d[:])
            nc.sync.dma_start(whqb[:], WhqB_d[:])
            icols = 128
            ident = const.tile([128, icols], F16, tag="ident")
            nc.sync.dma_start(ident[:], ident_d[:, 0:icols])
            if f16T:
                identT = ident
            else:
                identT = const.tile([128, H], F32, tag="identf")
                nc.vector.tensor_copy(identT[:], ident[:, 0:H])
            bo_row = const.tile([1, D], F16, tag="bo")
            ones4 = const.tile([1, NB], F16, tag="ones4")
            nc.sync.dma_start(bo_row[:], bo_d[:])
            nc.sync.dma_start(ones4[:], ones_d[:])
            # TComb col layout: c*32 + h*4 + b
            tcomb = const.tile([128, ET * H * NB], F16, tag="tcomb")

            # per-batch state handed to the (pipelined) epilogue
            state = {}

            def emit_stage(b):
                tet = inp.tile([128, ET * LT], F16, tag="tet", name="tet")
                iet = inp.tile([128, ET * LI], F16, tag="iet", name="iet")
                iau = inp.tile([128, YT * (D + 4)], F16, tag="iau", name="iau")
                tau = taugp.tile([128, XT * (D + 4)], F16, tag="taug", name="tau")
                if dma_multi:
                    nc.sync.dma_start(
                        tet[:].rearrange("p (j x) -> p j x", j=ET),
                        textT[b].rearrange("(j p) x -> p j x", p=128))
                    nc.sync.dma_start(
                        iet[:].rearrange("p (j y) -> p j y", j=ET),
                        imageT[b].rearrange("(j p) y -> p j y", p=128))
                    nc.sync.dma_start(
                        iau[:, 0:4 * (D + 4)].rearrange("p (j f) -> p j f", j=4),
                        image_aug[b, 0:512, :].rearrange("(j p) f -> p j f", p=128))
                    nc.sync.dma_start(iau[0:64, 4 * (D + 4):], image_aug[b, 512:576, :])
                    nc.sync.dma_start(
                        tau[:].rearrange("p (j f) -> p j f", j=XT),
                        text_aug[b].rearrange("(j p) f -> p j f", p=128))
                else:
                    for j in range(ET):
                        nc.sync.dma_start(tet[:, j * LT:(j + 1) * LT],
                                          textT[b, j * 128:(j + 1) * 128, :])
                        nc.sync.dma_start(iet[:, j * LI:(j + 1) * LI],
                                          imageT[b, j * 128:(j + 1) * 128, :])
                    for j in range(YT):
                        p = ycols(j)
                        nc.sync.dma_start(iau[0:p, j * (D + 4):(j + 1) * (D + 4)],
                                          image_aug[b, j * 128:j * 128 + p, :])
                    for j in range(XT):
                        nc.sync.dma_start(tau[:, j * (D + 4):(j + 1) * (D + 4)],
                                          text_aug[b, j * 128:(j + 1) * 128, :])
                state[b] = {"tet": tet, "iet": iet, "iau": iau, "tau": tau}

            def emit_exp_v(b):
                st = state[b]
                evf = perb.tile([128, YT * H], F16, tag="evf", name="evf", bufs=2)
                nc.scalar.activation(evf[:], st["sv"][:], AF.Exp)
                st["evf"] = evf

            def emit_exp_q(b):
                st = state[b]
                eqf = perb.tile([128, XT * H], F16, tag="eqf", name="eqf", bufs=2)
                nc.scalar.activation(eqf[:], st["sq"][:], AF.Exp)
                st["eqf"] = eqf

            def emit_epilogue_v(b):
                """ctx_v for batch b (uses iau tiles of batch b)."""
                st = state[b]
                st["ho"] = ho = perb.tile([H, D], F16 if f16T else F32, tag="ho", name="ho")
                _emit_ctx(st["evf"], YT, ho, st["iau"])

            def emit_epilogue_q(b):
                """ctx_q + combine for batch b (transposes emitted later)."""
                st = state[b]
                tmph = perb.tile([H, D], F16 if f16T else F32, tag="tmph", name="tmph")
                _emit_ctx(st["eqf"], XT, tmph, st["tau"])
                ho = st["ho"]
                nc.vector.tensor_tensor(out=ho[:], in0=ho[:], in1=tmph[:], op=OP.add)

            def emit_transposes(b):
                ho = state[b]["ho"]
                tc3 = tcomb[:].rearrange("p (c h b) -> p c h b", c=ET, h=H)
                for c in range(ET):
                    pst = psA.tile([128, H], F16 if f16T else F32, tag="mm1", name="pst")
                    nc.tensor.transpose(pst[:], ho[:, c * 128:(c + 1) * 128], identT[0:H, 0:H])
                    nc.vector.tensor_copy(tc3[:, c, :, b], pst[:])

            def _emit_ctx(er, nt, dest, aug):
                c512 = psA.tile([H, 512], F32, tag="mm1", name="c512")
                c257 = psA.tile([H, 260], F32, tag="mm1", name="c257")
                for j in range(nt):
                    p = ycols(j) if nt == YT else 128
                    ch = aug[0:p, j * (D + 4):(j + 1) * (D + 4)]
                    lhs = er[0:p, j * H:(j + 1) * H]
                    nc.tensor.matmul(c512[:], lhs, ch[:, 0:512],
                                     start=(j == 0), stop=(j == nt - 1))
                    nc.tensor.matmul(c257[:], lhs, ch[:, 512:772],
                                     start=(j == 0), stop=(j == nt - 1))
                rcp = perb.tile([H, 1], F32, tag="rcp", bufs=2, name="rcp")
                nc.vector.reciprocal(rcp[:], c257[:, 256:257])
                nc.vector.tensor_scalar_mul(dest[:, 0:512], c512[:], rcp[:])
                nc.vector.tensor_scalar_mul(dest[:, 512:768], c257[:, 0:256], rcp[:])

            import contextlib
            loop_cm = tc.For_i(0, repeat, 1) if repeat > 1 else contextlib.nullcontext()
            with loop_cm:
              emit_stage(0)
              for b in range(NB):
                  tet = state[b]["tet"]
                  iet = state[b]["iet"]
                  iau = state[b]["iau"]

                  # ---- wq_q [x,k] ----
                  wqq = perb.tile([128, XT * K], F16, tag="wqq", name="wqq")
                  for i in range(XT):
                      ps = psA.tile([128, K], F32, tag="mm1", name="psq")
                      for j in range(ET):
                          nc.tensor.matmul(
                              ps[:], tet[:, j * LT + i * 128: j * LT + (i + 1) * 128],
                              wqt[:, j * K:(j + 1) * K],
                              start=(j == 0), stop=(j == ET - 1))
                      nc.scalar.copy(wqq[:, i * K:(i + 1) * K], ps[:])

                  # ---- wv_v [y,k] ----
                  wvv = perb.tile([128, YT * K], F16, tag="wvv", name="wvv")
                  for j_y in range(YT):
                      p = ycols(j_y)
                      ps = psA.tile([128, K], F32, tag="mm1", name="psv")
                      for j in range(ET):
                          nc.tensor.matmul(
                              ps[0:p, :],
                              iet[:, j * LI + j_y * 128: j * LI + j_y * 128 + p],
                              wvt[:, j * K:(j + 1) * K],
                              start=(j == 0), stop=(j == ET - 1))
                      nc.scalar.copy(wvv[0:p, j_y * K:(j_y + 1) * K], ps[0:p, :])

                  if b > 0:
                      emit_epilogue_v(b - 1)

                  # ---- G_v [f,k] = image.T @ wv_v ----
                  gv = perb.tile([128, ET * K], F16, tag="gv", name="gv")
                  for c in range(ET):
                      ps = psA.tile([128, K], F32, tag="mm1", name="psg")
                      for j_y in range(YT):
                          p = ycols(j_y)
                          nc.tensor.matmul(
                              ps[:],
                              iau[0:p, j_y * (D + 4) + c * 128: j_y * (D + 4) + (c + 1) * 128],
                              wvv[0:p, j_y * K:(j_y + 1) * K],
                              start=(j_y == 0), stop=(j_y == YT - 1))
                      nc.scalar.copy(gv[:, c * K:(c + 1) * K], ps[:])

                  if b > 0:
                      emit_epilogue_q(b - 1)
                      emit_transposes(b - 1)

                  sv = sxp.tile([128, YT * H], F32, tag="sv", name="sv")
                  sq = sxp.tile([128, XT * H], F32, tag="sq", name="sq")
                  nc.vector.memset(sv[:], 0.0)
                  nc.vector.memset(sq[:], 0.0)
                  state[b]["sv"], state[b]["sq"] = sv, sq

                  # ---- heads ----
                  for h in range(H):
                      if h == H - 2 and b + 1 < NB:
                          emit_stage(b + 1)
                      wbt = wbtp.tile([128, ET * D], F16, tag="wbt", name="wbt")
                      if dma_multi:
                          nc.sync.dma_start(
                              wbt[:].rearrange("p (j f) -> p j f", j=ET),
                              WbT_d[h].rearrange("(j p) f -> p j f", p=128))
                      else:
                          for j in range(ET):
                              nc.sync.dma_start(wbt[:, j * D:(j + 1) * D],
                                                WbT_d[h, j * 128:(j + 1) * 128, :])

                      # ptT [f,x] = Wb[h] @ text.T
                      ptt = ptp.tile([128, ET * LT], F16, tag="ptt", name="ptt")
                      for c in range(ET):
                          ps = psA.tile([128, LT], F32, tag="mm1", name="psp")
                          for j in range(ET):
                              nc.tensor.matmul(
                                  ps[:], wbt[:, j * D + c * 128: j * D + (c + 1) * 128],
                                  tet[:, j * LT:(j + 1) * LT],
                                  start=(j == 0), stop=(j == ET - 1))
                          if ptt_eng == "split":
                              nc.vector.tensor_copy(
                                  ptt[:, c * LT: c * LT + 256], ps[:, 0:256])
                              nc.scalar.copy(
                                  ptt[:, c * LT + 256:(c + 1) * LT], ps[:, 256:512])
                          else:
                              ce = (("dve", "act")[c % 2]
                                    if ptt_eng == "rr" else ptt_eng)
                              copy_eng(ce)(ptt[:, c * LT:(c + 1) * LT], ps[:])

                      # aff [x,y] = pt @ image.T   (y split 288+288 across 2 banks)
                      aft = affp.tile([128, XT * LI], F16, tag="aff", name="aft")
                      for i in range(XT):
                          ps = psB.tile([128, 1024], F32, tag="aff", name="psaf")
                          for j in range(ET):
                              lhs = ptt[:, j * LT + i * 128: j * LT + (i + 1) * 128]
                              nc.tensor.matmul(
                                  ps[:, 0:288], lhs,
                                  iet[:, j * LI: j * LI + 288],
                                  start=(j == 0), stop=(j == ET - 1))
                              nc.tensor.matmul(
                                  ps[:, 512:800], lhs,
                                  iet[:, j * LI + 288: j * LI + 576],
                                  start=(j == 0), stop=(j == ET - 1))
                          ps3 = ps[:].rearrange("p (two x) -> p two x", two=2)[:, :, 0:288]
                          af3 = aft[:, i * LI:(i + 1) * LI].rearrange("p (two x) -> p two x", two=2)
                          copy_eng(aff_eng)(af3, ps3)

                      def s_chain(ps, p, addend, mms, whb, acc, amode=None):
                          """psum = addend + sum(mms); tanh; (.*whb) reduce -> acc."""
                          amode = amode or add_eng
                          if amode == "pe":
                              nc.tensor.matmul(ps, ident[0:p, 0:p], addend,
                                               start=True, stop=False)
                          for n, (lhs, rhs) in enumerate(mms):
                              nc.tensor.matmul(
                                  ps, lhs, rhs,
                                  start=(n == 0) and amode != "pe",
                                  stop=(n == len(mms) - 1))
                          if amode != "pe":
                              eng = nc.vector if amode == "dve" else nc.gpsimd
                              eng.tensor_tensor(out=ps, in0=ps, in1=addend, op=OP.add)
                          hc = hchp.tile([128, K], F16, tag="hch", name="hc")
                          nc.scalar.activation(hc[0:p, :], ps, AF.Tanh)
                          if use_ttr in (False, "mix"):
                              meng = nc.gpsimd if use_ttr == "mix" else nc.vector
                              pd = hchp.tile([128, K], F16, tag="scr", name="pd")
                              meng.tensor_tensor(
                                  out=pd[0:p, :], in0=hc[0:p, :], in1=whb[0:p, :],
                                  op=OP.mult)
                              nc.vector.tensor_reduce(
                                  acc, pd[0:p, :],
                                  axis=mybir.AxisListType.X, op=OP.add)
                          elif use_ttr == "oop":
                              pd = hchp.tile([128, K], F16, tag="scr", name="pd")
                              nc.vector.tensor_tensor_reduce(
                                  out=pd[0:p, :], in0=hc[0:p, :], in1=whb[0:p, :],
                                  scale=1.0, scalar=0.0,
                                  op0=OP.mult, op1=OP.add, accum_out=acc)
                          elif use_ttr:
                              nc.vector.tensor_tensor_reduce(
                                  out=hc[0:p, :], in0=hc[0:p, :], in1=whb[0:p, :],
                                  scale=1.0, scalar=0.0,
                                  op0=OP.mult, op1=OP.add, accum_out=acc)


                      def emit_sq(amode=None):
                          # S_q: needs ptt+gv+wqq only (covers aff copy latency)
                          for i in range(XT):
                              ps = psC.tile([128, K], F32, tag="pre", name="psc")
                              s_chain(
                                  ps[:], 128, wqq[:, i * K:(i + 1) * K],
                                  [(ptt[:, c * LT + i * 128: c * LT + (i + 1) * 128],
                                    gv[:, c * K:(c + 1) * K]) for c in range(ET)],
                                  whqb, sq[:, i * H + h: i * H + h + 1],
                                  amode=amode)

                      def emit_sv(amode=None):
                          # S_v: psum = wv_v + aff.T @ wq_q
                          for j_y in range(YT):
                              p = ycols(j_y)
                              ps = psC.tile([128, K], F32, tag="pre", name="psc")
                              s_chain(
                                  ps[0:p, :], p, wvv[0:p, j_y * K:(j_y + 1) * K],
                                  [(aft[:, i * LI + j_y * 128: i * LI + j_y * 128 + p],
                                    wqq[:, i * K:(i + 1) * K]) for i in range(XT)],
                                  whvb, sv[0:p, j_y * H + h: j_y * H + h + 1],
                                  amode=amode)

                      # last head: S_v first so the sv->exp drain hides under
                      # S_q's PE work (the next batch's epilogue reads evf)
                      if h == H - 1:
                          emit_sv(amode="pe")
                          emit_exp_v(b)
                          emit_sq(amode="pe")
                      else:
                          emit_sq(); emit_sv()
                  emit_exp_q(b)

              # prefetch first final-projection weight tiles, then drain
              wo_pre = []
              for tt in range(0, 8, 2):
                  wo = wotp.tile([128, 2 * D], F16, tag="wot", name="wo")
                  weng = nc.scalar if wo_eng == "act" else nc.sync
                  if dma_multi:
                      weng.dma_start(
                          wo[:].rearrange("p (t f) -> p t f", t=2),
                          WoT_d[tt * 128:(tt + 2) * 128, :].rearrange(
                              "(t p) f -> p t f", p=128))
                  else:
                      for t in (tt, tt + 1):
                          weng.dma_start(wo[:, (t - tt) * D:(t - tt + 1) * D],
                                         WoT_d[t * 128:(t + 1) * 128, :])
                  wo_pre.append(wo)

              # drain epilogue for the last batch
              emit_epilogue_v(NB - 1)
              emit_epilogue_q(NB - 1)
              emit_transposes(NB - 1)

              # ---- final: out = Wo @ combined + bo ----
              out_t = perb.tile([NB, D], F32, tag="outt", name="out_t")
              f512 = psA.tile([NB, 512], F32, tag="mm1", name="f512")
              f256 = psA.tile([NB, 256], F32, tag="mm1", name="f256")
              nc.tensor.matmul(f512[:], ones4[:], bo_row[:, 0:512],
                               start=True, stop=False)
              nc.tensor.matmul(f256[:], ones4[:], bo_row[:, 512:768],
                               start=True, stop=False)
              tc3 = tcomb[:].rearrange("p (c h b) -> p c h b", c=ET, h=H)
              for tt in range(0, H * ET, 2):
                  if tt < 8:
                      wo = wo_pre[tt // 2]
                  else:
                      wo = wotp.tile([128, 2 * D], F16, tag="wot", name="wo")
                      weng = nc.scalar if wo_eng == "act" else nc.sync
                      if dma_multi:
                          weng.dma_start(
                              wo[:].rearrange("p (t f) -> p t f", t=2),
                              WoT_d[tt * 128:(tt + 2) * 128, :].rearrange(
                                  "(t p) f -> p t f", p=128))
                      else:
                          for t in (tt, tt + 1):
                              weng.dma_start(wo[:, (t - tt) * D:(t - tt + 1) * D],
                                             WoT_d[t * 128:(t + 1) * 128, :])
                  for t in (tt, tt + 1):
                      h, c = t // ET, t % ET
                      woc = wo[:, (t - tt) * D:(t - tt + 1) * D]
                      lhs = tc3[:, c, h, :]
                      nc.tensor.matmul(f512[:], lhs, woc[:, 0:512],
                                       start=False, stop=(t == H * ET - 1))
                      nc.tensor.matmul(f256[:], lhs, woc[:, 512:768],
                                       start=False, stop=(t == H * ET - 1))
              nc.vector.tensor_copy(out_t[:, 0:512], f512[:])
              nc.vector.tensor_copy(out_t[:, 512:768], f256[:])
              nc.sync.dma_start(out_d[:], out_t[:])

    nc.compile()
    return nc


_nc_cache = None


def _get_nc():
    global _nc_cache
    if _nc_cache is None:
        _nc_cache = build_nc()
    return _nc_cache


def make_in_maps(inputs):
    return _make_in_maps(**inputs)


def _make_in_maps(text_hidden_states, image_hidden_states, text_mask, Wb, Wv,
                  Wq, Whv, Whq, Wo, bo, **_unused):
    f16 = np.float16
    text = np.asarray(text_hidden_states, np.float32)
    image = np.asarray(image_hidden_states, np.float32)
    Wb = np.asarray(Wb, np.float32)
    Wv = np.asarray(Wv, np.float32)
    Wq = np.asarray(Wq, np.float32)
    Whv = np.asarray(Whv, np.float32)
    Whq = np.asarray(Whq, np.float32)
    Wo = np.asarray(Wo, np.float32)
    bo = np.asarray(bo, np.float32)

    WqT = np.ascontiguousarray(Wq.T).astype(f16)
    WvT = np.ascontiguousarray(Wv.T).astype(f16)
    WbT = np.ascontiguousarray(np.transpose(Wb, (0, 2, 1))).astype(f16)
    WoT = np.ascontiguousarray(Wo.T).astype(f16)
    WhvB = np.ascontiguousarray(np.broadcast_to(Whv[None, :], (128, K))).astype(f16)
    WhqB = np.ascontiguousarray(np.broadcast_to(Whq[None, :], (128, K))).astype(f16)
    ident = np.eye(128, dtype=f16)

    textT = np.ascontiguousarray(np.transpose(text, (0, 2, 1))).astype(f16)
    imageT = np.ascontiguousarray(np.transpose(image, (0, 2, 1))).astype(f16)
    pad_t = np.zeros((B, LT, 4), np.float32); pad_t[:, :, 0] = 1.0
    pad_i = np.zeros((B, LI, 4), np.float32); pad_i[:, :, 0] = 1.0
    text_aug = np.concatenate([text, pad_t], axis=2).astype(f16)
    image_aug = np.concatenate([image, pad_i], axis=2).astype(f16)

    in_maps = []
    for c in range(N_CORES):
        sl = slice(c * NB, (c + 1) * NB)
        in_maps.append({
            "textT": textT[sl], "text_aug": text_aug[sl],
            "imageT": imageT[sl], "image_aug": image_aug[sl],
            "WqT": WqT, "WvT": WvT, "WbT": WbT,
            "WhvB": WhvB, "WhqB": WhqB, "WoT": WoT,
            "ident": ident,
            "bo_rep": bo[None, :].astype(f16),
            "ones4": np.ones((1, NB), f16),
        })
    return in_maps


def kernel(**inputs):
    nc = _get_nc()
    in_maps = make_in_maps(inputs)
    r = run_bass_kernel_spmd(nc, in_maps, list(range(N_CORES)))
    return np.concatenate([r.results[c]["out"] for c in range(N_CORES)], axis=0)


# revision 4
# speedup vs baseline: 1.0582x; 1.0217x over previous
"""Trainium2 Bass kernel for nn_MultiHeadCoAttention — v2 (fp16).

B=32, LT=512, LI=576, D=768, H=8, K=512. Batch-parallel over 8 cores
(4 batches per core, no collectives).

v2 vs v1 (f32r + gpsimd rounding everywhere):
- All matmul operands in fp16 (10-bit mantissa, full PE rate, DMA-native
  so no rounding passes; half the DMA traffic and SBUF of f32).
- tanh argument built in PSUM (matmul accumulation + DVE add), tanh on
  the scalar engine reading PSUM directly, writing fp16.
- (tanh * Whv) -> sum fused into one DVE tensor_tensor_reduce (fp16,
  2x DVE mode).
- PSUM->SBUF copies spread across DVE / ACT / Pool engines.
- Batched multi-tile DMAs; software-pipelined per-batch epilogue
  (softmax/ctx/transpose emitted under the next batch's matmuls);
  next-batch staging prefetched during the previous batch's heads.
"""
import sys
sys.path.insert(0, '/opt/trn_rl_repo')
import numpy as np
import concourse.bacc as bacc
import concourse.tile as tile
from concourse import bass, mybir
from concourse.bass_utils import run_bass_kernel_spmd

F32 = mybir.dt.float32
F16 = mybir.dt.float16
AF = mybir.ActivationFunctionType
OP = mybir.AluOpType

B, LT, LI, D, H, K = 32, 512, 576, 768, 8, 512
NB = 4           # batches per core
N_CORES = 8
ET = D // 128    # 6 e-tiles
XT = LT // 128   # 4 x-tiles
YT = 5           # y-tiles (4 full + 1 of 64)


def ycols(j):
    return 128 if j < 4 else 64


def build_nc(repeat=1, add_eng="dve", aff_eng="act", ptt_eng="act",
             psc_bufs=3, psb_bufs=1, psa_bufs=3, hch_bufs=3, wo_eng="sync", f16T=True, dma_multi=True,
             use_ttr=False):
    nc = bacc.Bacc(None, target_bir_lowering=False)

    # ---- DRAM I/O (per core), all fp16 except the f32 output ----
    textT = nc.dram_tensor("textT", [NB, D, LT], F16, kind="ExternalInput")
    text_aug = nc.dram_tensor("text_aug", [NB, LT, D + 4], F16, kind="ExternalInput")
    imageT = nc.dram_tensor("imageT", [NB, D, LI], F16, kind="ExternalInput")
    image_aug = nc.dram_tensor("image_aug", [NB, LI, D + 4], F16, kind="ExternalInput")
    WqT_d = nc.dram_tensor("WqT", [D, K], F16, kind="ExternalInput")
    WvT_d = nc.dram_tensor("WvT", [D, K], F16, kind="ExternalInput")
    WbT_d = nc.dram_tensor("WbT", [H, D, D], F16, kind="ExternalInput")
    WhvB_d = nc.dram_tensor("WhvB", [128, K], F16, kind="ExternalInput")
    WhqB_d = nc.dram_tensor("WhqB", [128, K], F16, kind="ExternalInput")
    WoT_d = nc.dram_tensor("WoT", [H * D, D], F16, kind="ExternalInput")
    ident_d = nc.dram_tensor("ident", [128, 128], F16, kind="ExternalInput")
    bo_d = nc.dram_tensor("bo_rep", [1, D], F16, kind="ExternalInput")
    ones_d = nc.dram_tensor("ones4", [1, NB], F16, kind="ExternalInput")
    out_d = nc.dram_tensor("out", [NB, D], F32, kind="ExternalOutput")

    with tile.TileContext(nc) as tc:
        with (
            tc.tile_pool(name="const", bufs=1) as const,
            tc.tile_pool(name="inp", bufs=2) as inp,
            tc.tile_pool(name="perb", bufs=1) as perb,
            tc.tile_pool(name="sxp", bufs=2) as sxp,        # sv/sq
            tc.tile_pool(name="taugp", bufs=2) as taugp,    # text_aug, resident
            tc.tile_pool(name="wbt", bufs=2) as wbtp,
            tc.tile_pool(name="ptp", bufs=2) as ptp,
            tc.tile_pool(name="affp", bufs=2) as affp,
            tc.tile_pool(name="hch", bufs=hch_bufs) as hchp,
            tc.tile_pool(name="wot", bufs=4) as wotp,
            tc.tile_pool(name="psA", bufs=psa_bufs, space="PSUM") as psA,
            tc.tile_pool(name="psB", bufs=psb_bufs, space="PSUM") as psB,
            tc.tile_pool(name="psC", bufs=psc_bufs, space="PSUM") as psC,
        ):
            def copy_eng(eng):
                if eng == "dve":
                    return nc.vector.tensor_copy
                if eng == "act":
                    return nc.scalar.copy
                return nc.gpsimd.tensor_copy

            # ---- constants (outside the timing loop) ----
            wqt = const.tile([128, ET * K], F16, tag="wqt")
            wvt = const.tile([128, ET * K], F16, tag="wvt")
            if dma_multi:
                nc.sync.dma_start(
                    wqt[:].rearrange("p (j k) -> p j k", j=ET),
                    WqT_d[:].rearrange("(j p) k -> p j k", p=128))
                nc.sync.dma_start(
                    wvt[:].rearrange("p (j k) -> p j k", j=ET),
                    WvT_d[:].rearrange("(j p) k -> p j k", p=128))
            else:
                for j in range(ET):
                    nc.sync.dma_start(wqt[:, j * K:(j + 1) * K],
                                      WqT_d[j * 128:(j + 1) * 128, :])
                    nc.sync.dma_start(wvt[:, j * K:(j + 1) * K],
                                      WvT_d[j * 128:(j + 1) * 128, :])
            whvb = const.tile([128, K], F16, tag="whvb")
            whqb = const.tile([128, K], F16, tag="whqb")
            nc.sync.dma_start(whvb[:], WhvB_d[:])
            nc.sync.dma_start(whqb[:], WhqB_d[:])
            icols = 128
            ident = const.tile([128, icols], F16, tag="ident")
            nc.sync.dma_start(ident[:], ident_d[:, 0:icols])
            if f16T:
                identT = ident
            else:
                identT = const.tile([128, H], F32, tag="identf")
                nc.vector.tensor_copy(identT[:], ident[:, 0:H])
            bo_row = const.tile([1, D], F16, tag="bo")
            ones4 = const.tile([1, NB], F16, tag="ones4")
            nc.sync.dma_start(bo_row[:], bo_d[:])
            nc.sync.dma_start(ones4[:], ones_d[:])
            # TComb col layout: c*32 + h*4 + b
            tcomb = const.tile([128, ET * H * NB], F16, tag="tcomb")

            # per-batch state handed to the (pipelined) epilogue
            state = {}

            def emit_stage(b):
                tet = inp.tile([128, ET * LT], F16, tag="tet", name="tet")
                iet = inp.tile([128, ET * LI], F16, tag="iet", name="iet")
                iau = inp.tile([128, YT * (D + 4)], F16, tag="iau", name="iau")
                tau = taugp.tile([128, XT * (D + 4)], F16, tag="taug", name="tau")
                if dma_multi:
                    nc.sync.dma_start(
                        tet[:].rearrange("p (j x) -> p j x", j=ET),
                        textT[b].rearrange("(j p) x -> p j x", p=128))
                    nc.sync.dma_start(
                        iet[:].rearrange("p (j y) -> p j y", j=ET),
                        imageT[b].rearrange("(j p) y -> p j y", p=128))
                    nc.sync.dma_start(
                        iau[:, 0:4 * (D + 4)].rearrange("p (j f) -> p j f", j=4),
                        image_aug[b, 0:512, :].rearrange("(j p) f -> p j f", p=128))
                    nc.sync.dma_start(iau[0:64, 4 * (D + 4):], image_aug[b, 512:576, :])
                    nc.sync.dma_start(
                        tau[:].rearrange("p (j f) -> p j f", j=XT),
                        text_aug[b].rearrange("(j p) f -> p j f", p=128))
                else:
                    for j in range(ET):
                        nc.sync.dma_start(tet[:, j * LT:(j + 1) * LT],
                                          textT[b, j * 128:(j + 1) * 128, :])
                        nc.sync.dma_start(iet[:, j * LI:(j + 1) * LI],
                                          imageT[b, j * 128:(j + 1) * 128, :])
                    for j in range(YT):
                        p = ycols(j)
                        nc.sync.dma_start(iau[0:p, j * (D + 4):(j + 1) * (D + 4)],
                                          image_aug[b, j * 128:j * 128 + p, :])
                    for j in range(XT):
                        nc.sync.dma_start(tau[:, j * (D + 4):(j + 1) * (D + 4)],
                                          text_aug[b, j * 128:(j + 1) * 128, :])
                state[b] = {"tet": tet, "iet": iet, "iau": iau, "tau": tau}

            def emit_exp_v(b):
                st = state[b]
                evf = perb.tile([128, YT * H], F16, tag="evf", name="evf", bufs=2)
                nc.scalar.activation(evf[:], st["sv"][:], AF.Exp)
                st["evf"] = evf

            def emit_exp_q(b):
                st = state[b]
                eqf = perb.tile([128, XT * H], F16, tag="eqf", name="eqf", bufs=2)
                nc.scalar.activation(eqf[:], st["sq"][:], AF.Exp)
                st["eqf"] = eqf

            def emit_epilogue_v(b):
                """ctx_v for batch b (uses iau tiles of batch b)."""
                st = state[b]
                st["ho"] = ho = perb.tile([H, D], F16 if f16T else F32, tag="ho", name="ho")
                _emit_ctx(st["evf"], YT, ho, st["iau"])

            def emit_epilogue_q(b):
                """ctx_q + combine for batch b (transposes emitted later)."""
                st = state[b]
                tmph = perb.tile([H, D], F16 if f16T else F32, tag="tmph", name="tmph")
                _emit_ctx(st["eqf"], XT, tmph, st["tau"])
                ho = st["ho"]
                nc.vector.tensor_tensor(out=ho[:], in0=ho[:], in1=tmph[:], op=OP.add)

            def emit_transposes(b):
                ho = state[b]["ho"]
                tc3 = tcomb[:].rearrange("p (c h b) -> p c h b", c=ET, h=H)
                for c in range(ET):
                    pst = psA.tile([128, H], F16 if f16T else F32, tag="mm1", name="pst")
                    nc.tensor.transpose(pst[:], ho[:, c * 128:(c + 1) * 128], identT[0:H, 0:H])
                    nc.vector.tensor_copy(tc3[:, c, :, b], pst[:])

            def _emit_ctx(er, nt, dest, aug):
                c512 = psA.tile([H, 512], F32, tag="mm1", name="c512")
                c257 = psA.tile([H, 260], F32, tag="mm1", name="c257")
                for j in range(nt):
                    p = ycols(j) if nt == YT else 128
                    ch = aug[0:p, j * (D + 4):(j + 1) * (D + 4)]
                    lhs = er[0:p, j * H:(j + 1) * H]
                    nc.tensor.matmul(c512[:], lhs, ch[:, 0:512],
                                     start=(j == 0), stop=(j == nt - 1))
                    nc.tensor.matmul(c257[:], lhs, ch[:, 512:772],
                                     start=(j == 0), stop=(j == nt - 1))
                rcp = perb.tile([H, 1], F32, tag="rcp", bufs=2, name="rcp")
                nc.vector.reciprocal(rcp[:], c257[:, 256:257])
                nc.vector.tensor_scalar_mul(dest[:, 0:512], c512[:], rcp[:])
                nc.vector.tensor_scalar_mul(dest[:, 512:768], c257[:, 0:256], rcp[:])

            import contextlib
            loop_cm = tc.For_i(0, repeat, 1) if repeat > 1 else contextlib.nullcontext()
            with loop_cm:
              emit_stage(0)
              for b in range(NB):
                  tet = state[b]["tet"]
                  iet = state[b]["iet"]
                  iau = state[b]["iau"]

                  # ---- wq_q [x,k] ----
                  wqq = perb.tile([128, XT * K], F16, tag="wqq", name="wqq")
                  for i in range(XT):
                      ps = psA.tile([128, K], F32, tag="mm1", name="psq")
                      for j in range(ET):
                          nc.tensor.matmul(
                              ps[:], tet[:, j * LT + i * 128: j * LT + (i + 1) * 128],
                              wqt[:, j * K:(j + 1) * K],
                              start=(j == 0), stop=(j == ET - 1))
                      nc.scalar.copy(wqq[:, i * K:(i + 1) * K], ps[:])

                  # ---- wv_v [y,k] ----
                  wvv = perb.tile([128, YT * K], F16, tag="wvv", name="wvv")
                  for j_y in range(YT):
                      p = ycols(j_y)
                      ps = psA.tile([128, K], F32, tag="mm1", name="psv")
                      for j in range(ET):
                          nc.tensor.matmul(
                              ps[0:p, :],
                              iet[:, j * LI + j_y * 128: j * LI + j_y * 128 + p],
                              wvt[:, j * K:(j + 1) * K],
                              start=(j == 0), stop=(j == ET - 1))
                      nc.scalar.copy(wvv[0:p, j_y * K:(j_y + 1) * K], ps[0:p, :])

                  if b > 0:
                      emit_epilogue_v(b - 1)

                  # ---- G_v [f,k] = image.T @ wv_v ----
                  gv = perb.tile([128, ET * K], F16, tag="gv", name="gv")
                  for c in range(ET):
                      ps = psA.tile([128, K], F32, tag="mm1", name="psg")
                      for j_y in range(YT):
                          p = ycols(j_y)
                          nc.tensor.matmul(
                              ps[:],
                              iau[0:p, j_y * (D + 4) + c * 128: j_y * (D + 4) + (c + 1) * 128],
                              wvv[0:p, j_y * K:(j_y + 1) * K],
                              start=(j_y == 0), stop=(j_y == YT - 1))
                      nc.scalar.copy(gv[:, c * K:(c + 1) * K], ps[:])

                  if b > 0:
                      emit_epilogue_q(b - 1)
                      emit_transposes(b - 1)

                  sv = sxp.tile([128, YT * H], F32, tag="sv", name="sv")
                  sq = sxp.tile([128, XT * H], F32, tag="sq", name="sq")
                  nc.vector.memset(sv[:], 0.0)
                  nc.vector.memset(sq[:], 0.0)
                  state[b]["sv"], state[b]["sq"] = sv, sq

                  # ---- heads ----
                  for h in range(H):
                      if h == H - 2 and b + 1 < NB:
                          emit_stage(b + 1)
                      wbt = wbtp.tile([128, ET * D], F16, tag="wbt", name="wbt")
                      if dma_multi:
                          nc.sync.dma_start(
                              wbt[:].rearrange("p (j f) -> p j f", j=ET),
                              WbT_d[h].rearrange("(j p) f -> p j f", p=128))
                      else:
                          for j in range(ET):
                              nc.sync.dma_start(wbt[:, j * D:(j + 1) * D],
                                                WbT_d[h, j * 128:(j + 1) * 128, :])

                      # ptT [f,x] = Wb[h] @ text.T
                      ptt = ptp.tile([128, ET * LT], F16, tag="ptt", name="ptt")
                      for c in range(ET):
                          ps = psA.tile([128, LT], F32, tag="mm1", name="psp")
                          for j in range(ET):
                              nc.tensor.matmul(
                                  ps[:], wbt[:, j * D + c * 128: j * D + (c + 1) * 128],
                                  tet[:, j * LT:(j + 1) * LT],
                                  start=(j == 0), stop=(j == ET - 1))
                          if ptt_eng == "split":
                              nc.vector.tensor_copy(
                                  ptt[:, c * LT: c * LT + 256], ps[:, 0:256])
                              nc.scalar.copy(
                                  ptt[:, c * LT + 256:(c + 1) * LT], ps[:, 256:512])
                          else:
                              ce = (("dve", "act")[c % 2]
                                    if ptt_eng == "rr" else ptt_eng)
                              copy_eng(ce)(ptt[:, c * LT:(c + 1) * LT], ps[:])

                      # aff [x,y] = pt @ image.T   (y split 288+288 across 2 banks)
                      aft = affp.tile([128, XT * LI], F16, tag="aff", name="aft")
                      for i in range(XT):
                          ps = psB.tile([128, 1024], F32, tag="aff", name="psaf")
                          for j in range(ET):
                              lhs = ptt[:, j * LT + i * 128: j * LT + (i + 1) * 128]
                              nc.tensor.matmul(
                                  ps[:, 0:288], lhs,
                                  iet[:, j * LI: j * LI + 288],
                                  start=(j == 0), stop=(j == ET - 1))
                              nc.tensor.matmul(
                                  ps[:, 512:800], lhs,
                                  iet[:, j * LI + 288: j * LI + 576],
                                  start=(j == 0), stop=(j == ET - 1))
                          ps3 = ps[:].rearrange("p (two x) -> p two x", two=2)[:, :, 0:288]
                          af3 = aft[:, i * LI:(i + 1) * LI].rearrange("p (two x) -> p two x", two=2)
                          copy_eng(aff_eng)(af3, ps3)

                      def s_chain(ps, p, addend, mms, whb, acc, amode=None):
                          """psum = addend + sum(mms); tanh; (.*whb) reduce -> acc."""
                          amode = amode or add_eng
                          if amode == "pe":
                              nc.tensor.matmul(ps, ident[0:p, 0:p], addend,
                                               start=True, stop=False)
                          for n, (lhs, rhs) in enumerate(mms):
                              nc.tensor.matmul(
                                  ps, lhs, rhs,
                                  start=(n == 0) and amode != "pe",
                                  stop=(n == len(mms) - 1))
                          if amode != "pe":
                              eng = nc.vector if amode == "dve" else nc.gpsimd
                              eng.tensor_tensor(out=ps, in0=ps, in1=addend, op=OP.add)
                          hc = hchp.tile([128, K], F16, tag="hch", name="hc")
                          nc.scalar.activation(hc[0:p, :], ps, AF.Tanh)
                          if use_ttr in (False, "mix"):
                              meng = nc.gpsimd if use_ttr == "mix" else nc.vector
                              pd = hchp.tile([128, K], F16, tag="scr", name="pd")
                              meng.tensor_tensor(
                                  out=pd[0:p, :], in0=hc[0:p, :], in1=whb[0:p, :],
                                  op=OP.mult)
                              nc.vector.tensor_reduce(
                                  acc, pd[0:p, :],
                                  axis=mybir.AxisListType.X, op=OP.add)
                          elif use_ttr == "oop":
                              pd = hchp.tile([128, K], F16, tag="scr", name="pd")
                              nc.vector.tensor_tensor_reduce(
                                  out=pd[0:p, :], in0=hc[0:p, :], in1=whb[0:p, :],
                                  scale=1.0, scalar=0.0,
                                  op0=OP.mult, op1=OP.add, accum_out=acc)
                          elif use_ttr:
                              nc.vector.tensor_tensor_reduce(
                                  out=hc[0:p, :], in0=hc[0:p, :], in1=whb[0:p, :],
                                  scale=1.0, scalar=0.0,
                                  op0=OP.mult, op1=OP.add, accum_out=acc)


                      def emit_sq(amode=None):
                          # S_q: needs ptt+gv+wqq only (covers aff copy latency)
                          for i in range(XT):
                              ps = psC.tile([128, K], F32, tag="pre", name="psc")
                              s_chain(
                                  ps[:], 128, wqq[:, i * K:(i + 1) * K],
                                  [(ptt[:, c * LT + i * 128: c * LT + (i + 1) * 128],
                                    gv[:, c * K:(c + 1) * K]) for c in range(ET)],
                                  whqb, sq[:, i * H + h: i * H + h + 1],
                                  amode=amode)

                      def emit_sv(amode=None):
                          # S_v: psum = wv_v + aff.T @ wq_q
                          for j_y in range(YT):
                              p = ycols(j_y)
                              ps = psC.tile([128, K], F32, tag="pre", name="psc")
                              s_chain(
                                  ps[0:p, :], p, wvv[0:p, j_y * K:(j_y + 1) * K],
                                  [(aft[:, i * LI + j_y * 128: i * LI + j_y * 128 + p],
                                    wqq[:, i * K:(i + 1) * K]) for i in range(XT)],
                                  whvb, sv[0:p, j_y * H + h: j_y * H + h + 1],
                                  amode=amode)

                      # last head: S_v first so the sv->exp drain hides under
                      # S_q's PE work (the next batch's epilogue reads evf)
                      if h == H - 1:
                          emit_sv(amode="pe")
                          emit_exp_v(b)
                          emit_sq(amode="pe")
                      else:
                          emit_sq(); emit_sv()
                  emit_exp_q(b)

              # prefetch first final-projection weight tiles, then drain
              wo_pre = []
              for tt in range(0, 8, 2):
                  wo = wotp.tile([128, 2 * D], F16, tag="wot", name="wo")
                  weng = nc.scalar if wo_eng == "act" else nc.sync
                  if dma_multi:
                      weng.dma_start(
                          wo[:].rearrange("p (t f) -> p t f", t=2),
                          WoT_d[tt * 128:(tt + 2) * 128, :].rearrange(
                              "(t p) f -> p t f", p=128))
                  else:
                      for t in (tt, tt + 1):
                          weng.dma_start(wo[:, (t - tt) * D:(t - tt + 1) * D],
                                         WoT_d[t * 128:(t + 1) * 128, :])
                  wo_pre.append(wo)

              # drain epilogue for the last batch
              emit_epilogue_v(NB - 1)
              emit_epilogue_q(NB - 1)
              emit_transposes(NB - 1)

              # ---- final: out = Wo @ combined + bo ----
              out_t = perb.tile([NB, D], F32, tag="outt", name="out_t")
              f512 = psA.tile([NB, 512], F32, tag="mm1", name="f512")
              f256 = psA.tile([NB, 256], F32, tag="mm1", name="f256")
              nc.tensor.matmul(f512[:], ones4[:], bo_row[:, 0:512],
                               start=True, stop=False)
              nc.tensor.matmul(f256[:], ones4[:], bo_row[:, 512:768],
                               start=True, stop=False)
              tc3 = tcomb[:].rearrange("p (c h b) -> p c h b", c=ET, h=H)
              for tt in range(0, H * ET, 2):
                  if tt < 8:
                      wo = wo_pre[tt // 2]
                  else:
                      wo = wotp.tile([128, 2 * D], F16, tag="wot", name="wo")
                      weng = nc.scalar if wo_eng == "act" else nc.sync
                      if dma_multi:
                          weng.dma_start(
                              wo[:].rearrange("p (t f) -> p t f", t=2),
                              WoT_d[tt * 128:(tt + 2) * 128, :].rearrange(
                                  "(t p) f -> p t f", p=128))
                      else:
                          for t in (tt, tt + 1):
                              weng.dma_start(wo[:, (t - tt) * D:(t - tt + 1) * D],
                                             WoT_d[t * 128:(t + 1) * 128, :])
                  for t in (tt, tt + 1):
                      h, c = t // ET, t % ET
                      woc = wo[:, (t - tt) * D:(t - tt + 1) * D]
                      lhs = tc3[:, c, h, :]
                      nc.tensor.matmul(f512[:], lhs, woc[:, 0:512],
                                       start=False, stop=(t == H * ET - 1))
                      nc.tensor.matmul(f256[:], lhs, woc[:, 512:768],
                                       start=False, stop=(t == H * ET - 1))
              nc.vector.tensor_copy(out_t[:, 0:512], f512[:])
              nc.vector.tensor_copy(out_t[:, 512:768], f256[:])
              nc.sync.dma_start(out_d[:], out_t[:])

    nc.compile()
    return nc


_nc_cache = None


def _get_nc():
    global _nc_cache
    if _nc_cache is None:
        _nc_cache = build_nc()
    return _nc_cache


def make_in_maps(inputs):
    return _make_in_maps(**inputs)


def _make_in_maps(text_hidden_states, image_hidden_states, text_mask, Wb, Wv,
                  Wq, Whv, Whq, Wo, bo, **_unused):
    f16 = np.float16
    text = np.asarray(text_hidden_states, np.float32)
    image = np.asarray(image_hidden_states, np.float32)
    Wb = np.asarray(Wb, np.float32)
    Wv = np.asarray(Wv, np.float32)
    Wq = np.asarray(Wq, np.float32)
    Whv = np.asarray(Whv, np.float32)
    Whq = np.asarray(Whq, np.float32)
    Wo = np.asarray(Wo, np.float32)
    bo = np.asarray(bo, np.float32)

    WqT = np.ascontiguousarray(Wq.T).astype(f16)
    WvT = np.ascontiguousarray(Wv.T).astype(f16)
    WbT = np.ascontiguousarray(np.transpose(Wb, (0, 2, 1))).astype(f16)
    WoT = np.ascontiguousarray(Wo.T).astype(f16)
    WhvB = np.ascontiguousarray(np.broadcast_to(Whv[None, :], (128, K))).astype(f16)
    WhqB = np.ascontiguousarray(np.broadcast_to(Whq[None, :], (128, K))).astype(f16)
    ident = np.eye(128, dtype=f16)

    textT = np.ascontiguousarray(np.transpose(text, (0, 2, 1))).astype(f16)
    imageT = np.ascontiguousarray(np.transpose(image, (0, 2, 1))).astype(f16)
    pad_t = np.zeros((B, LT, 4), np.float32); pad_t[:, :, 0] = 1.0
    pad_i = np.zeros((B, LI, 4), np.float32); pad_i[:, :, 0] = 1.0
    text_aug = np.concatenate([text, pad_t], axis=2).astype(f16)
    image_aug = np.concatenate([image, pad_i], axis=2).astype(f16)

    in_maps = []
    for c in range(N_CORES):
        sl = slice(c * NB, (c + 1) * NB)
        in_maps.append({
            "textT": textT[sl], "text_aug": text_aug[sl],
            "imageT": imageT[sl], "image_aug": image_aug[sl],
            "WqT": WqT, "WvT": WvT, "WbT": WbT,
            "WhvB": WhvB, "WhqB": WhqB, "WoT": WoT,
            "ident": ident,
            "bo_rep": bo[None, :].astype(f16),
            "ones4": np.ones((1, NB), f16),
        })
    return in_maps


def kernel(**inputs):
    nc = _get_nc()
    in_maps = make_in_maps(inputs)
    r = run_bass_kernel_spmd(nc, in_maps, list(range(N_CORES)))
    return np.concatenate([r.results[c]["out"] for c in range(N_CORES)], axis=0)
